# revision 31
# baseline (speedup 1.0000x reference)
"""Trainium2 Bass kernel for the 3-layer spiking neural network (DSNN).

Strategy (v2)
-------------
Data-parallel over batch: 256 rows / 8 cores = 32 per core, weights
replicated, zero collectives. The timestep loop is restructured so every
engine stays busy and the PE (the hard floor at ~1.28us/step of f32r
matmul) is the pacemaker instead of the DVE:

  1. Spike train S (binary) generated on-device (DVE compare, f32r out)
     in feature-major layout, bias row folded in as an Act bias add.
  2. H0 = S @ W0 per 9-step block (single m11 pass - W0 is rounded to
     e8m11 on host; measured end-to-end rel-l2 vs fp32 reference 9.4e-3,
     under the 2e-2 gate).
  3. Membrane recurrences store the PRE-reset membrane z (decode
     select(z>thr, 0, z) happens inside the next step's update), which
     fuses spike+reset+decay+drive into ONE custom DVE op per step -
     and layers 0 and 1 are packed side by side in one [128, 512] tile
     so both layers cost a single instruction (z1 runs 2 blocks behind
     z0 so its drive is ready).
  4. mm1 consumes Q_t = alpha*Q_{t-1} + s0_t (alpha-scanned spikes,
     custom DVE op) instead of raw spikes, so H1 = Q @ W1 produces the
     layer-1 synaptic state y1 directly (exact by linearity) - the
     per-step y1 AXPY disappears; Act copies mm1 PSUM straight into the
     drive tile.
  5. abar = sum_t w_t * (z1_t > thr) accumulates on the Pool engine
     (tensor_scalar is_gt*w then tensor_tensor add), w_t the closed-form
     alpha/beta decay weight. mem2 = abar @ W2 exactly (layer 2 never
     resets), one small matmul at the end.

Per-step engine budget (Tb=9): PE 1.28us (mm0 32MM + mm1 64MM, f32r,
N=288), DVE 1.27us (fused-z 690ns + qgen 424ns + spike-gen), Pool
1.04us (abar), Act 0.68us (PSUM->SBUF copies). Weight DMAs ride the
Act HWDGE queue, RT streams ride the SP queue.
"""

import numpy as np

ALPHA = 0.9
BETA = 0.85
THR = 1.0
T = 99            # timesteps actually simulated (t = 1..99 of 100)
BCORE = 32        # batch per core
NCORES = 8
TB = 9            # timesteps per block; 99 = 11 * 9 exactly
NB = T // TB
NK = TB * BCORE   # matmul free dim per block

_CACHE = {}


def _register_custom_ops():
    """Fused SNN custom-DVE ops (runtime-registered; the per-NEFF DVE
    table is generated from OPS at compile time).

    SNN_MEM: z' = select(z > s1, 0, z) * s0 + drive   (decode + decay + drive)
    SNN_QGEN: q' = (z > s1) + qprev * s0              (alpha-scanned spikes)
    """
    import concourse.dve_ops as dve_ops
    if "SNN_MEM" in dve_ops._SUB_OPCODE_FOR_NAME:
        return (next(o for o in dve_ops.OPS if o.name == "SNN_MEM"),
                next(o for o in dve_ops.OPS if o.name == "SNN_QGEN"),
                next(o for o in dve_ops.OPS if o.name == "SNN_ABARW"))
    from concourse.dve_spec import (
        Spec, Src0, Src1, Zero, select, lower, _has_src1, C0, C1, C2)
    from concourse.dve_uop import DveOpSpec

    def make(name, spec):
        row = dve_ops._CUSTOM_DVE_ROW_BASE + len(dve_ops.OPS)
        assert row < 0x20
        dve_ops._SUB_OPCODE_FOR_NAME[name] = row
        shas = {}
        for ver in ("v3", "v4"):
            uops = lower(spec, ver=ver)
            shas[ver] = DveOpSpec(name=name, opcode=row, uops=uops,
                                  rd1_en=_has_src1(spec)).sha(ver)
        op = dve_ops.DveOp(name, spec, subdim=False, uops_sha=shas)
        dve_ops.OPS.append(op)
        dve_ops.CUSTOM_DVE_SPECS[name] = spec
        return op

    f32 = np.float32
    mem = make("SNN_MEM", Spec(
        body=select(Src0 > C1, Zero, Src0) * C0 + Src1,
        reference=lambda in0, in1, s0, s1, imm2:
            (np.where(in0 > f32(s1), f32(0.0), in0) * f32(s0) + in1).astype(f32),
    ))
    qgen = make("SNN_QGEN", Spec(
        body=(Src0 > C1) + Src1 * C0,
        reference=lambda in0, in1, s0, s1, imm2:
            ((in0 > f32(s1)).astype(f32) + in1 * f32(s0)).astype(f32),
    ))
    abarw = make("SNN_ABARW", Spec(
        body=select(Src0 > C1, C2, Zero) + Src1,
        reference=lambda in0, in1, s0, s1, imm2:
            (np.where(in0 > f32(s1), f32(imm2), f32(0.0)) + in1).astype(f32),
    ))
    return mem, qgen, abarw


def _round_m11(x):
    # hw float32r = e8m11, round-to-nearest on the 12 dropped bits
    xi = np.ascontiguousarray(np.asarray(x, np.float32)).view(np.uint32).astype(np.uint64)
    bias = np.uint64(0x7FF) + ((xi >> np.uint64(12)) & np.uint64(1))
    return ((xi + bias) & np.uint64(0xFFFFF000)).astype(np.uint32).view(np.float32)


def _decay_weights():
    # w_j = sum_{k=0}^{T-1-j} BETA^(T-1-j-k) * ALPHA^k
    w = np.zeros(T, np.float64)
    for j in range(T):
        n = T - 1 - j
        k = np.arange(n + 1)
        w[j] = np.sum(BETA ** (n - k) * (ALPHA ** k))
    return w.astype(np.float32)


def build_program():
    """Build + compile the (SPMD, per-core) Bass program once."""
    if "nc" in _CACHE:
        return _CACHE["nc"]
    import concourse.bacc as bacc
    import concourse.mybir as mybir
    import concourse.tile as tile

    f32 = mybir.dt.float32
    f32r = mybir.dt.float32r
    A = mybir.AluOpType
    Act = mybir.ActivationFunctionType

    OP_MEM, OP_QGEN, OP_ABARW = _register_custom_ops()
    W = _decay_weights()
    SIGK = 16384.0

    nc = bacc.Bacc("TRN2", target_bir_lowering=False, debug=False,
                   enable_asserts=False, num_devices=NCORES)

    RT = nc.dram_tensor("RT", [512, T * BCORE], f32, kind="ExternalInput").ap()
    xT = nc.dram_tensor("xT", [512, BCORE], f32, kind="ExternalInput").ap()
    W0d = nc.dram_tensor("W0d", [512, 1024], f32r, kind="ExternalInput").ap()
    W1d = nc.dram_tensor("W1d", [1024, 1024], f32r, kind="ExternalInput").ap()
    W2d = nc.dram_tensor("W2d", [1024, 512], f32r, kind="ExternalInput").ap()
    b0d = nc.dram_tensor("b0d", [128, 8], f32, kind="ExternalInput").ap()
    Wtd = nc.dram_tensor("Wtd", [128, T], f32, kind="ExternalInput").ap()
    outd = nc.dram_tensor("out", [BCORE, 512], f32, kind="ExternalOutput").ap()

    with tile.TileContext(nc) as tc:
        with (
            tc.tile_pool(name="const", bufs=1) as cpool,
            tc.tile_pool(name="rt", bufs=2) as rt_pool,
            tc.tile_pool(name="sblk", bufs=2) as s_pool,
            tc.tile_pool(name="dd", bufs=3) as d_pool,
            tc.tile_pool(name="zz", bufs=2) as z_pool,
            tc.tile_pool(name="qq", bufs=2) as q_pool,
            tc.tile_pool(name="tmp", bufs=4) as tmp_pool,
            tc.tile_pool(name="ps", bufs=7, space="PSUM") as ps_pool,
            tc.tile_pool(name="psf", bufs=1, space="PSUM") as psf_pool,
        ):
            # ---- constants / weights ----
            w0_sb = cpool.tile([128, 4 * 1024], f32r, tag="w0")
            w1_sb = cpool.tile([128, 8 * 1024], f32r, tag="w1")
            w2_sb = cpool.tile([128, 8 * 512], f32r, tag="w2")
            b0_sb = cpool.tile([128, 8], f32, tag="b0")
            xt_sb = cpool.tile([128, 4 * BCORE], f32, tag="xt")

            # All input DMAs ride the SP queue, ordered so nothing gates the
            # pipeline: xt/b0 (tiny) -> rt(0) -> W0 (mm0(0)) -> rt(1) ->
            # W1 (mm1(0), needed an iteration later) -> W2 (needed at the
            # end).  Keeping the Act queue free of DMA issue lets the first
            # h0 copies (and thus fused(0)) start as soon as mm0(0) lands.
            nc.sync.dma_start(
                out=xt_sb[:].rearrange("p (c b) -> p c b", c=4),
                in_=xT.rearrange("(c p) b -> p c b", p=128))
            nc.sync.dma_start(out=b0_sb[:], in_=b0d)

            # ---- persistent state ----
            abar = cpool.tile([128, 256], f32, tag="abar")
            zinit = cpool.tile([128, 512], f32, tag="zinit")
            negk = cpool.tile([128, 1], f32, tag="negk")
            wfull = cpool.tile([128, T], f32, tag="wfull")
            nc.vector.memset(abar[:], 0.0)
            nc.vector.memset(zinit[:], 0.0)
            nc.vector.memset(negk[:], -SIGK)
            nc.sync.dma_start(out=wfull[:], in_=Wtd)

            rt4 = RT.rearrange("(c p) n -> p c n", p=128)
            sblk_t, rt_t, Z_t, Q_t, D_t = {}, {}, {}, {}, {}

            def rt_dma(k):
                rt = rt_pool.tile([128, 4 * NK], f32, tag="rt")
                for c in range(4):
                    nc.sync.dma_start(
                        out=rt[:, c * NK:(c + 1) * NK],
                        in_=rt4[:, c, k * NK:(k + 1) * NK])
                rt_t[k] = rt

            def stage_sg(k):
                # binary spike block: S = (x > r), f32r, [128, (c t b)]
                sblk = s_pool.tile([128, 4 * NK], f32r, tag="sblk")
                rt = rt_t.pop(k)
                xc = (xt_sb[:].rearrange("p (c b) -> p c b", c=4)
                      .unsqueeze(2).broadcast_to([128, 4, TB, BCORE]))
                ssl = sblk[:].rearrange("p (c t b) -> p c t b", c=4, t=TB)
                rsl = rt[:].rearrange("p (c t b) -> p c t b", c=4, t=TB)
                nc.vector.tensor_tensor(out=ssl, in0=xc, in1=rsl, op=A.is_gt)
                sblk_t[k] = sblk

            def get_D(k):
                if k not in D_t:
                    D_t[k] = d_pool.tile([128, TB * 512], f32, tag="dd",
                                         name=f"dd{k}")
                return D_t[k]

            def stage_mm0(k):
                # H0 = S @ W0 (single m11 pass) -> D_k lower halves + bias
                sblk = sblk_t.pop(k)
                D = get_D(k)
                dv = D[:].rearrange("p (t x) -> p t x", t=TB)
                for c in range(8):
                    ps = ps_pool.tile([128, NK], f32, tag="ps")
                    for kc in range(4):
                        nc.tensor.matmul(
                            ps[:],
                            lhsT=w0_sb[:, kc * 1024 + c * 128: kc * 1024 + (c + 1) * 128],
                            rhs=sblk[:, kc * NK:(kc + 1) * NK],
                            start=(kc == 0), stop=(kc == 3))
                    nc.scalar.activation(
                        out=dv[:, :, c * BCORE:(c + 1) * BCORE],
                        in_=ps[:].rearrange("p (t b) -> p t b", t=TB),
                        func=Act.Identity, bias=b0_sb[:, c:c + 1], scale=1.0)

            def stage_fused(k):
                # one custom op per step updates BOTH membranes [z0|z1]
                Z = z_pool.tile([128, TB * 512], f32, tag="zz")
                Zv = Z[:].rearrange("p (t x) -> p t x", t=TB)
                D = get_D(k)
                Dv = D[:].rearrange("p (t x) -> p t x", t=TB)
                Zp = Z_t.pop(k - 1, None)
                prev = (zinit[:] if Zp is None
                        else Zp[:].rearrange("p (t x) -> p t x", t=TB)[:, TB - 1, :])
                for i in range(TB):
                    nc.vector._custom_dve(OP_MEM, out=Zv[:, i, :], in0=prev,
                                          in1=Dv[:, i, :], s0=BETA, s1=THR)
                    prev = Zv[:, i, :]
                Z_t[k] = Z

            def stage_qgen(k):
                # Q_t = alpha*Q_{t-1} + (z0_t > thr)  (f32r, mm1 rhs)
                Q = q_pool.tile([128, TB * 256], f32r, tag="qq")
                Qv = Q[:].rearrange("p (t x) -> p t x", t=TB)
                Zv = Z_t[k][:].rearrange("p (t x) -> p t x", t=TB)
                Qp = Q_t.pop(k - 1, None)
                prev = (zinit[:, 0:256] if Qp is None
                        else Qp[:].rearrange("p (t x) -> p t x", t=TB)[:, TB - 1, :])
                for i in range(TB):
                    nc.vector._custom_dve(OP_QGEN, out=Qv[:, i, :],
                                          in0=Zv[:, i, 0:256], in1=prev,
                                          s0=ALPHA, s1=THR)
                    prev = Qv[:, i, :]
                Q_t[k] = Q

            def stage_mm1(k):
                # y1 = Q @ W1 -> D_{k+2} upper halves (drive for z1)
                Q = Q_t[k]
                Qv = Q[:].rearrange("p (t c b) -> p c t b", t=TB, c=8)
                D = get_D(k + 2)
                dv = D[:].rearrange("p (t x) -> p t x", t=TB)
                for c in range(8):
                    ps = ps_pool.tile([128, NK], f32, tag="ps")
                    for kc in range(8):
                        nc.tensor.matmul(
                            ps[:],
                            lhsT=w1_sb[:, kc * 1024 + c * 128: kc * 1024 + (c + 1) * 128],
                            rhs=Qv[:, kc],
                            start=(kc == 0), stop=(kc == 7))
                    nc.scalar.activation(
                        out=dv[:, :, 256 + c * BCORE: 256 + (c + 1) * BCORE],
                        in_=ps[:].rearrange("p (t b) -> p t b", t=TB),
                        func=Act.Copy)

            def stage_abar(zv_src, j, dve_only=False):
                # abar += w_t * (z1 > thr).  Step 0 runs as one fused DVE op;
                # the rest extract the spike on Act (saturated sigmoid step)
                # and multiply/accumulate with two Pool tensor_tensor ops,
                # keeping the DVE at/below the PE budget.
                for i in range(TB):
                    if dve_only or i == 0:
                        nc.vector._custom_dve(
                            OP_ABARW, out=abar[:], in0=zv_src[:, i, 256:512],
                            in1=abar[:], s1=THR, imm2=float(W[j * TB + i]))
                        continue
                    s1 = tmp_pool.tile([128, 256], f32, tag="tmp")
                    nc.scalar.activation(out=s1[:], in_=zv_src[:, i, 256:512],
                                         func=Act.Sigmoid, scale=SIGK,
                                         bias=negk[:, 0:1])
                    s1w = tmp_pool.tile([128, 256], f32, tag="tmp")
                    nc.gpsimd.tensor_tensor(
                        out=s1w[:], in0=s1[:],
                        in1=wfull[:, j * TB + i: j * TB + i + 1]
                            .broadcast_to([128, 256]),
                        op=A.mult)
                    nc.gpsimd.tensor_tensor(out=abar[:], in0=abar[:],
                                            in1=s1w[:], op=A.add)

            # ---- prologue ----
            # memset upper (y1) halves of D_0/D_1: z1 runs 2 blocks behind,
            # so its first 2 blocks of drive are zero.
            for k in (0, 1):
                D = get_D(k)
                dv = D[:].rearrange("p (t x) -> p t x", t=TB)
                nc.vector.memset(dv[:, :, 256:512], 0.0)
            rt_dma(0)
            nc.sync.dma_start(
                out=w0_sb[:].rearrange("p (k m) -> p k m", k=4),
                in_=W0d.rearrange("(k p) m -> p k m", p=128))
            rt_dma(1)
            nc.sync.dma_start(
                out=w1_sb[:].rearrange("p (k m) -> p k m", k=8),
                in_=W1d.rearrange("(k p) m -> p k m", p=128))
            nc.sync.dma_start(
                out=w2_sb[:].rearrange("p (k m) -> p k m", k=8),
                in_=W2d.rearrange("(k p) m -> p k m", p=128))
            stage_sg(0)
            stage_mm0(0)

            # ---- main loop ----
            for k in range(NB):
                if k + 2 < NB:
                    rt_dma(k + 2)
                if k + 1 < NB:
                    stage_sg(k + 1)
                    stage_mm0(k + 1)
                # mm1 lags one block so its rhs (Q) is ready when the PE
                # gets to it - no mid-iteration PE stall on the DVE.
                if k >= 1:
                    stage_mm1(k - 1)
                stage_fused(k)
                stage_qgen(k)
                if k == NB - 1:
                    # last mm1 goes out right behind qgen so the PE never
                    # idles between the main loop and the epilogue.
                    stage_mm1(k)
                if k >= 2:
                    stage_abar(
                        Z_t[k][:].rearrange("p (t x) -> p t x", t=TB), k - 2)

            # ---- epilogue: z1 for blocks NB-2, NB-1 ----
            for e in (NB, NB + 1):
                Z = z_pool.tile([128, TB * 512], f32, tag="zz")
                Zv = Z[:].rearrange("p (t x) -> p t x", t=TB)
                D = get_D(e)
                Dv = D[:].rearrange("p (t x) -> p t x", t=TB)
                Zp = Z_t.pop(e - 1)
                prev = Zp[:].rearrange("p (t x) -> p t x", t=TB)[:, TB - 1, 256:512]
                for i in range(TB):
                    nc.vector._custom_dve(OP_MEM, out=Zv[:, i, 256:512],
                                          in0=prev, in1=Dv[:, i, 256:512],
                                          s0=BETA, s1=THR)
                    prev = Zv[:, i, 256:512]
                Z_t[e] = Z
                stage_abar(Zv, e - 2, dve_only=True)

            # ---- final: mem2 = abar @ W2 ----
            af = cpool.tile([128, 256], f32r, tag="af")
            nc.vector.tensor_copy(af[:], abar[:])
            psf = psf_pool.tile([BCORE, 512], f32, tag="psf")
            for kc in range(8):
                nc.tensor.matmul(
                    psf[:],
                    lhsT=af[:, kc * BCORE:(kc + 1) * BCORE],
                    rhs=w2_sb[:, kc * 512:(kc + 1) * 512],
                    start=(kc == 0), stop=(kc == 7))
            outsb = cpool.tile([BCORE, 512], f32, tag="outsb")
            nc.scalar.activation(out=outsb[:], in_=psf[:], func=Act.Copy)
            nc.sync.dma_start(out=outd, in_=outsb[:])

    nc.compile()
    _CACHE["nc"] = nc
    return nc


def make_in_maps(inputs, W0, W1, W2, random_distribution):
    """Host-side shard prep: slice batch, transpose to feature-major,
    round weights to e8m11 for the f32r matmul path."""
    inputs = np.ascontiguousarray(np.asarray(inputs, np.float32))
    W0 = np.asarray(W0, np.float32)
    W1 = np.asarray(W1, np.float32)
    W2 = np.asarray(W2, np.float32)
    R = np.asarray(random_distribution, np.float32)

    W0r = np.ascontiguousarray(_round_m11(W0[:512]))
    W1r = np.ascontiguousarray(_round_m11(W1))
    W2r = np.ascontiguousarray(_round_m11(W2))
    b0 = np.ascontiguousarray(W0[512].reshape(8, 128).T)  # [128, 8]
    Wd = _decay_weights()
    Wt = np.ascontiguousarray(np.broadcast_to(Wd[None, :], (128, T)))

    in_maps = []
    for i in range(NCORES):
        sl = slice(i * BCORE, (i + 1) * BCORE)
        xTi = np.ascontiguousarray(inputs[sl].T)  # [512, 32]
        # [99, 32, 512] -> [512, 99*32] feature-major
        RTi = np.ascontiguousarray(
            R[1:, sl, :512].transpose(2, 0, 1).reshape(512, T * BCORE))
        in_maps.append({
            "RT": RTi, "xT": xTi, "W0d": W0r,
            "W1d": W1r, "W2d": W2r, "b0d": b0, "Wtd": Wt,
        })
    return in_maps


def kernel(inputs, W0, W1, W2, random_distribution):
    from concourse.bass_utils import run_bass_kernel_spmd
    nc = build_program()
    in_maps = make_in_maps(inputs, W0, W1, W2, random_distribution)
    res = run_bass_kernel_spmd(nc, in_maps, core_ids=list(range(NCORES)))
    outs = [np.asarray(res.results[i]["out"], np.float32) for i in range(NCORES)]
    return np.concatenate(outs, axis=0)


if __name__ == "__main__":
    d = np.load("/tmp/snn_inputs.npz")
    out = kernel(d["inputs"], d["W0"], d["W1"], d["W2"], d["random_distribution"])
    exp = d["expected"]
    rel = np.linalg.norm(out - exp) / np.linalg.norm(exp)
    print("kernel vs reference rel_l2:", rel)


# revision 33
# speedup vs baseline: 1.0041x; 1.0041x over previous
"""Trainium2 Bass kernel for the 3-layer spiking neural network (DSNN).

Strategy (v2)
-------------
Data-parallel over batch: 256 rows / 8 cores = 32 per core, weights
replicated, zero collectives. The timestep loop is restructured so every
engine stays busy and the PE (the hard floor at ~1.28us/step of f32r
matmul) is the pacemaker instead of the DVE:

  1. Spike train S (binary) generated on-device (DVE compare, f32r out)
     in feature-major layout, bias row folded in as an Act bias add.
  2. H0 = S @ W0 per 9-step block (single m11 pass - W0 is rounded to
     e8m11 on host; measured end-to-end rel-l2 vs fp32 reference 9.4e-3,
     under the 2e-2 gate).
  3. Membrane recurrences store the PRE-reset membrane z (decode
     select(z>thr, 0, z) happens inside the next step's update), which
     fuses spike+reset+decay+drive into ONE custom DVE op per step -
     and layers 0 and 1 are packed side by side in one [128, 512] tile
     so both layers cost a single instruction (z1 runs 2 blocks behind
     z0 so its drive is ready).
  4. mm1 consumes Q_t = alpha*Q_{t-1} + s0_t (alpha-scanned spikes,
     custom DVE op) instead of raw spikes, so H1 = Q @ W1 produces the
     layer-1 synaptic state y1 directly (exact by linearity) - the
     per-step y1 AXPY disappears; Act copies mm1 PSUM straight into the
     drive tile.
  5. abar = sum_t w_t * (z1_t > thr) accumulates on the Pool engine
     (tensor_scalar is_gt*w then tensor_tensor add), w_t the closed-form
     alpha/beta decay weight. mem2 = abar @ W2 exactly (layer 2 never
     resets), one small matmul at the end.

Per-step engine budget (Tb=9): PE 1.28us (mm0 32MM + mm1 64MM, f32r,
N=288), DVE 1.27us (fused-z 690ns + qgen 424ns + spike-gen), Pool
1.04us (abar), Act 0.68us (PSUM->SBUF copies). Weight DMAs ride the
Act HWDGE queue, RT streams ride the SP queue.
"""

import numpy as np

ALPHA = 0.9
BETA = 0.85
THR = 1.0
T = 99            # timesteps actually simulated (t = 1..99 of 100)
BCORE = 32        # batch per core
NCORES = 8
TB = 9            # timesteps per block; 99 = 11 * 9 exactly
NB = T // TB
NK = TB * BCORE   # matmul free dim per block

_CACHE = {}


def _register_custom_ops():
    """Fused SNN custom-DVE ops (runtime-registered; the per-NEFF DVE
    table is generated from OPS at compile time).

    SNN_MEM: z' = select(z > s1, 0, z) * s0 + drive   (decode + decay + drive)
    SNN_QGEN: q' = (z > s1) + qprev * s0              (alpha-scanned spikes)
    """
    import concourse.dve_ops as dve_ops
    if "SNN_MEM" in dve_ops._SUB_OPCODE_FOR_NAME:
        return (next(o for o in dve_ops.OPS if o.name == "SNN_MEM"),
                next(o for o in dve_ops.OPS if o.name == "SNN_QGEN"),
                next(o for o in dve_ops.OPS if o.name == "SNN_ABARW"))
    from concourse.dve_spec import (
        Spec, Src0, Src1, Zero, select, lower, _has_src1, C0, C1, C2)
    from concourse.dve_uop import DveOpSpec

    def make(name, spec):
        row = dve_ops._CUSTOM_DVE_ROW_BASE + len(dve_ops.OPS)
        assert row < 0x20
        dve_ops._SUB_OPCODE_FOR_NAME[name] = row
        shas = {}
        for ver in ("v3", "v4"):
            uops = lower(spec, ver=ver)
            shas[ver] = DveOpSpec(name=name, opcode=row, uops=uops,
                                  rd1_en=_has_src1(spec)).sha(ver)
        op = dve_ops.DveOp(name, spec, subdim=False, uops_sha=shas)
        dve_ops.OPS.append(op)
        dve_ops.CUSTOM_DVE_SPECS[name] = spec
        return op

    f32 = np.float32
    mem = make("SNN_MEM", Spec(
        body=select(Src0 > C1, Zero, Src0) * C0 + Src1,
        reference=lambda in0, in1, s0, s1, imm2:
            (np.where(in0 > f32(s1), f32(0.0), in0) * f32(s0) + in1).astype(f32),
    ))
    qgen = make("SNN_QGEN", Spec(
        body=(Src0 > C1) + Src1 * C0,
        reference=lambda in0, in1, s0, s1, imm2:
            ((in0 > f32(s1)).astype(f32) + in1 * f32(s0)).astype(f32),
    ))
    abarw = make("SNN_ABARW", Spec(
        body=select(Src0 > C1, C2, Zero) + Src1,
        reference=lambda in0, in1, s0, s1, imm2:
            (np.where(in0 > f32(s1), f32(imm2), f32(0.0)) + in1).astype(f32),
    ))
    return mem, qgen, abarw


def _round_m11(x):
    # hw float32r = e8m11, round-to-nearest on the 12 dropped bits
    xi = np.ascontiguousarray(np.asarray(x, np.float32)).view(np.uint32).astype(np.uint64)
    bias = np.uint64(0x7FF) + ((xi >> np.uint64(12)) & np.uint64(1))
    return ((xi + bias) & np.uint64(0xFFFFF000)).astype(np.uint32).view(np.float32)


def _decay_weights():
    # w_j = sum_{k=0}^{T-1-j} BETA^(T-1-j-k) * ALPHA^k
    w = np.zeros(T, np.float64)
    for j in range(T):
        n = T - 1 - j
        k = np.arange(n + 1)
        w[j] = np.sum(BETA ** (n - k) * (ALPHA ** k))
    return w.astype(np.float32)


def build_program():
    """Build + compile the (SPMD, per-core) Bass program once."""
    if "nc" in _CACHE:
        return _CACHE["nc"]
    import concourse.bacc as bacc
    import concourse.mybir as mybir
    import concourse.tile as tile

    f32 = mybir.dt.float32
    f32r = mybir.dt.float32r
    A = mybir.AluOpType
    Act = mybir.ActivationFunctionType

    OP_MEM, OP_QGEN, OP_ABARW = _register_custom_ops()
    W = _decay_weights()
    SIGK = 16384.0

    nc = bacc.Bacc("TRN2", target_bir_lowering=False, debug=False,
                   enable_asserts=False, num_devices=NCORES)

    RT = nc.dram_tensor("RT", [512, T * BCORE], f32, kind="ExternalInput").ap()
    xT = nc.dram_tensor("xT", [512, BCORE], f32, kind="ExternalInput").ap()
    W0d = nc.dram_tensor("W0d", [512, 1024], f32r, kind="ExternalInput").ap()
    W1d = nc.dram_tensor("W1d", [1024, 1024], f32r, kind="ExternalInput").ap()
    W2d = nc.dram_tensor("W2d", [1024, 512], f32r, kind="ExternalInput").ap()
    b0d = nc.dram_tensor("b0d", [128, 8], f32, kind="ExternalInput").ap()
    Wtd = nc.dram_tensor("Wtd", [128, T], f32, kind="ExternalInput").ap()
    outd = nc.dram_tensor("out", [BCORE, 512], f32, kind="ExternalOutput").ap()

    with tile.TileContext(nc) as tc:
        with (
            tc.tile_pool(name="const", bufs=1) as cpool,
            tc.tile_pool(name="rt", bufs=2) as rt_pool,
            tc.tile_pool(name="sblk", bufs=3) as s_pool,
            tc.tile_pool(name="dd", bufs=3) as d_pool,
            tc.tile_pool(name="zz", bufs=2) as z_pool,
            tc.tile_pool(name="qq", bufs=2) as q_pool,
            tc.tile_pool(name="tmp", bufs=4) as tmp_pool,
            tc.tile_pool(name="ps", bufs=7, space="PSUM") as ps_pool,
            tc.tile_pool(name="psf", bufs=1, space="PSUM") as psf_pool,
        ):
            # ---- constants / weights ----
            w0_sb = cpool.tile([128, 4 * 1024], f32r, tag="w0")
            w1_sb = cpool.tile([128, 8 * 1024], f32r, tag="w1")
            w2_sb = cpool.tile([128, 8 * 512], f32r, tag="w2")
            b0_sb = cpool.tile([128, 8], f32, tag="b0")
            xt_sb = cpool.tile([128, 4 * BCORE], f32, tag="xt")

            # All input DMAs ride the SP queue, ordered so nothing gates the
            # pipeline: xt/b0 (tiny) -> rt(0) -> W0 (mm0(0)) -> rt(1) ->
            # W1 (mm1(0), needed an iteration later) -> W2 (needed at the
            # end).  Keeping the Act queue free of DMA issue lets the first
            # h0 copies (and thus fused(0)) start as soon as mm0(0) lands.
            nc.sync.dma_start(
                out=xt_sb[:].rearrange("p (c b) -> p c b", c=4),
                in_=xT.rearrange("(c p) b -> p c b", p=128))
            nc.sync.dma_start(out=b0_sb[:], in_=b0d)

            # ---- persistent state ----
            abar = cpool.tile([128, 256], f32, tag="abar")
            zinit = cpool.tile([128, 512], f32, tag="zinit")
            negk = cpool.tile([128, 1], f32, tag="negk")
            wfull = cpool.tile([128, T], f32, tag="wfull")
            nc.vector.memset(abar[:], 0.0)
            nc.vector.memset(zinit[:], 0.0)
            nc.vector.memset(negk[:], -SIGK)
            nc.sync.dma_start(out=wfull[:], in_=Wtd)

            rt4 = RT.rearrange("(c p) n -> p c n", p=128)
            sblk_t, rt_t, Z_t, Q_t, D_t = {}, {}, {}, {}, {}

            def rt_dma(k):
                rt = rt_pool.tile([128, 4 * NK], f32, tag="rt")
                for c in range(4):
                    nc.sync.dma_start(
                        out=rt[:, c * NK:(c + 1) * NK],
                        in_=rt4[:, c, k * NK:(k + 1) * NK])
                rt_t[k] = rt

            def stage_sg(k):
                # binary spike block: S = (x > r), f32r, [128, (c t b)]
                sblk = s_pool.tile([128, 4 * NK], f32r, tag="sblk")
                rt = rt_t.pop(k)
                xc = (xt_sb[:].rearrange("p (c b) -> p c b", c=4)
                      .unsqueeze(2).broadcast_to([128, 4, TB, BCORE]))
                ssl = sblk[:].rearrange("p (c t b) -> p c t b", c=4, t=TB)
                rsl = rt[:].rearrange("p (c t b) -> p c t b", c=4, t=TB)
                nc.vector.tensor_tensor(out=ssl, in0=xc, in1=rsl, op=A.is_gt)
                sblk_t[k] = sblk

            def get_D(k):
                if k not in D_t:
                    D_t[k] = d_pool.tile([128, TB * 512], f32, tag="dd",
                                         name=f"dd{k}")
                return D_t[k]

            def stage_mm0(k):
                # H0 = S @ W0 (single m11 pass) -> D_k lower halves + bias
                sblk = sblk_t.pop(k)
                D = get_D(k)
                dv = D[:].rearrange("p (t x) -> p t x", t=TB)
                for c in range(8):
                    ps = ps_pool.tile([128, NK], f32, tag="ps")
                    for kc in range(4):
                        nc.tensor.matmul(
                            ps[:],
                            lhsT=w0_sb[:, kc * 1024 + c * 128: kc * 1024 + (c + 1) * 128],
                            rhs=sblk[:, kc * NK:(kc + 1) * NK],
                            start=(kc == 0), stop=(kc == 3))
                    nc.scalar.activation(
                        out=dv[:, :, c * BCORE:(c + 1) * BCORE],
                        in_=ps[:].rearrange("p (t b) -> p t b", t=TB),
                        func=Act.Identity, bias=b0_sb[:, c:c + 1], scale=1.0)

            def stage_fused(k):
                # one custom op per step updates BOTH membranes [z0|z1]
                Z = z_pool.tile([128, TB * 512], f32, tag="zz")
                Zv = Z[:].rearrange("p (t x) -> p t x", t=TB)
                D = get_D(k)
                Dv = D[:].rearrange("p (t x) -> p t x", t=TB)
                Zp = Z_t.pop(k - 1, None)
                prev = (zinit[:] if Zp is None
                        else Zp[:].rearrange("p (t x) -> p t x", t=TB)[:, TB - 1, :])
                for i in range(TB):
                    nc.vector._custom_dve(OP_MEM, out=Zv[:, i, :], in0=prev,
                                          in1=Dv[:, i, :], s0=BETA, s1=THR)
                    prev = Zv[:, i, :]
                Z_t[k] = Z

            def stage_qgen(k):
                # Q_t = alpha*Q_{t-1} + (z0_t > thr)  (f32r, mm1 rhs)
                Q = q_pool.tile([128, TB * 256], f32r, tag="qq")
                Qv = Q[:].rearrange("p (t x) -> p t x", t=TB)
                Zv = Z_t[k][:].rearrange("p (t x) -> p t x", t=TB)
                Qp = Q_t.pop(k - 1, None)
                prev = (zinit[:, 0:256] if Qp is None
                        else Qp[:].rearrange("p (t x) -> p t x", t=TB)[:, TB - 1, :])
                for i in range(TB):
                    nc.vector._custom_dve(OP_QGEN, out=Qv[:, i, :],
                                          in0=Zv[:, i, 0:256], in1=prev,
                                          s0=ALPHA, s1=THR)
                    prev = Qv[:, i, :]
                Q_t[k] = Q

            def stage_mm1(k):
                # y1 = Q @ W1 -> D_{k+2} upper halves (drive for z1)
                Q = Q_t[k]
                Qv = Q[:].rearrange("p (t c b) -> p c t b", t=TB, c=8)
                D = get_D(k + 2)
                dv = D[:].rearrange("p (t x) -> p t x", t=TB)
                for c in range(8):
                    ps = ps_pool.tile([128, NK], f32, tag="ps")
                    for kc in range(8):
                        nc.tensor.matmul(
                            ps[:],
                            lhsT=w1_sb[:, kc * 1024 + c * 128: kc * 1024 + (c + 1) * 128],
                            rhs=Qv[:, kc],
                            start=(kc == 0), stop=(kc == 7))
                    nc.scalar.activation(
                        out=dv[:, :, 256 + c * BCORE: 256 + (c + 1) * BCORE],
                        in_=ps[:].rearrange("p (t b) -> p t b", t=TB),
                        func=Act.Copy)

            def stage_abar(zv_src, j, dve_only=False):
                # abar += w_t * (z1 > thr).  Step 0 runs as one fused DVE op;
                # the rest extract the spike on Act (saturated sigmoid step)
                # and multiply/accumulate with two Pool tensor_tensor ops,
                # keeping the DVE at/below the PE budget.
                for i in range(TB):
                    if dve_only or i == 0:
                        nc.vector._custom_dve(
                            OP_ABARW, out=abar[:], in0=zv_src[:, i, 256:512],
                            in1=abar[:], s1=THR, imm2=float(W[j * TB + i]))
                        continue
                    s1 = tmp_pool.tile([128, 256], f32, tag="tmp")
                    nc.scalar.activation(out=s1[:], in_=zv_src[:, i, 256:512],
                                         func=Act.Sigmoid, scale=SIGK,
                                         bias=negk[:, 0:1])
                    s1w = tmp_pool.tile([128, 256], f32, tag="tmp")
                    nc.gpsimd.tensor_tensor(
                        out=s1w[:], in0=s1[:],
                        in1=wfull[:, j * TB + i: j * TB + i + 1]
                            .broadcast_to([128, 256]),
                        op=A.mult)
                    nc.gpsimd.tensor_tensor(out=abar[:], in0=abar[:],
                                            in1=s1w[:], op=A.add)

            # ---- prologue ----
            # memset upper (y1) halves of D_0/D_1: z1 runs 2 blocks behind,
            # so its first 2 blocks of drive are zero.
            for k in (0, 1):
                D = get_D(k)
                dv = D[:].rearrange("p (t x) -> p t x", t=TB)
                nc.vector.memset(dv[:, :, 256:512], 0.0)
            rt_dma(0)
            nc.sync.dma_start(
                out=w0_sb[:].rearrange("p (k m) -> p k m", k=4),
                in_=W0d.rearrange("(k p) m -> p k m", p=128))
            rt_dma(1)
            nc.sync.dma_start(
                out=w1_sb[:].rearrange("p (k m) -> p k m", k=8),
                in_=W1d.rearrange("(k p) m -> p k m", p=128))
            nc.sync.dma_start(
                out=w2_sb[:].rearrange("p (k m) -> p k m", k=8),
                in_=W2d.rearrange("(k p) m -> p k m", p=128))
            stage_sg(0)
            stage_sg(1)
            stage_mm0(0)

            # ---- main loop ----
            for k in range(NB):
                if k + 2 < NB:
                    rt_dma(k + 2)
                if k + 1 < NB:
                    stage_mm0(k + 1)
                # mm1 lags one block so its rhs (Q) is ready when the PE
                # gets to it - no mid-iteration PE stall on the DVE.
                if k >= 1:
                    stage_mm1(k - 1)
                stage_fused(k)
                stage_qgen(k)
                # spike-gen two blocks ahead, at the tail of this
                # iteration's DVE queue, so mm0(k+1) never waits on it.
                if k + 2 < NB:
                    stage_sg(k + 2)
                if k == NB - 1:
                    # last mm1 goes out right behind qgen so the PE never
                    # idles between the main loop and the epilogue.
                    stage_mm1(k)
                if k >= 2:
                    stage_abar(
                        Z_t[k][:].rearrange("p (t x) -> p t x", t=TB), k - 2)

            # ---- epilogue: z1 for blocks NB-2, NB-1 ----
            for e in (NB, NB + 1):
                Z = z_pool.tile([128, TB * 512], f32, tag="zz")
                Zv = Z[:].rearrange("p (t x) -> p t x", t=TB)
                D = get_D(e)
                Dv = D[:].rearrange("p (t x) -> p t x", t=TB)
                Zp = Z_t.pop(e - 1)
                prev = Zp[:].rearrange("p (t x) -> p t x", t=TB)[:, TB - 1, 256:512]
                for i in range(TB):
                    nc.vector._custom_dve(OP_MEM, out=Zv[:, i, 256:512],
                                          in0=prev, in1=Dv[:, i, 256:512],
                                          s0=BETA, s1=THR)
                    prev = Zv[:, i, 256:512]
                Z_t[e] = Z
                stage_abar(Zv, e - 2, dve_only=True)

            # ---- final: mem2 = abar @ W2 ----
            af = cpool.tile([128, 256], f32r, tag="af")
            nc.vector.tensor_copy(af[:], abar[:])
            psf = psf_pool.tile([BCORE, 512], f32, tag="psf")
            for kc in range(8):
                nc.tensor.matmul(
                    psf[:],
                    lhsT=af[:, kc * BCORE:(kc + 1) * BCORE],
                    rhs=w2_sb[:, kc * 512:(kc + 1) * 512],
                    start=(kc == 0), stop=(kc == 7))
            outsb = cpool.tile([BCORE, 512], f32, tag="outsb")
            nc.scalar.activation(out=outsb[:], in_=psf[:], func=Act.Copy)
            nc.sync.dma_start(out=outd, in_=outsb[:])

    nc.compile()
    _CACHE["nc"] = nc
    return nc


def make_in_maps(inputs, W0, W1, W2, random_distribution):
    """Host-side shard prep: slice batch, transpose to feature-major,
    round weights to e8m11 for the f32r matmul path."""
    inputs = np.ascontiguousarray(np.asarray(inputs, np.float32))
    W0 = np.asarray(W0, np.float32)
    W1 = np.asarray(W1, np.float32)
    W2 = np.asarray(W2, np.float32)
    R = np.asarray(random_distribution, np.float32)

    W0r = np.ascontiguousarray(_round_m11(W0[:512]))
    W1r = np.ascontiguousarray(_round_m11(W1))
    W2r = np.ascontiguousarray(_round_m11(W2))
    b0 = np.ascontiguousarray(W0[512].reshape(8, 128).T)  # [128, 8]
    Wd = _decay_weights()
    Wt = np.ascontiguousarray(np.broadcast_to(Wd[None, :], (128, T)))

    in_maps = []
    for i in range(NCORES):
        sl = slice(i * BCORE, (i + 1) * BCORE)
        xTi = np.ascontiguousarray(inputs[sl].T)  # [512, 32]
        # [99, 32, 512] -> [512, 99*32] feature-major
        RTi = np.ascontiguousarray(
            R[1:, sl, :512].transpose(2, 0, 1).reshape(512, T * BCORE))
        in_maps.append({
            "RT": RTi, "xT": xTi, "W0d": W0r,
            "W1d": W1r, "W2d": W2r, "b0d": b0, "Wtd": Wt,
        })
    return in_maps


def kernel(inputs, W0, W1, W2, random_distribution):
    from concourse.bass_utils import run_bass_kernel_spmd
    nc = build_program()
    in_maps = make_in_maps(inputs, W0, W1, W2, random_distribution)
    res = run_bass_kernel_spmd(nc, in_maps, core_ids=list(range(NCORES)))
    outs = [np.asarray(res.results[i]["out"], np.float32) for i in range(NCORES)]
    return np.concatenate(outs, axis=0)


if __name__ == "__main__":
    d = np.load("/tmp/snn_inputs.npz")
    out = kernel(d["inputs"], d["W0"], d["W1"], d["W2"], d["random_distribution"])
    exp = d["expected"]
    rel = np.linalg.norm(out - exp) / np.linalg.norm(exp)
    print("kernel vs reference rel_l2:", rel)
